# revision 4
# baseline (speedup 1.0000x reference)
"""Trainium2 Bass kernel for nn_AttentionHead (B=32, C=256, H=W=32).

Reference computation (per batch b):
    xs = x[b].reshape(C, S).T                     # [S, C], S = H*W = 1024
    q = xs @ wq.T + bq ; k = xs @ wk.T + bk ; v = xs @ wv.T + bv
    attn = softmax(q @ k.T / sqrt(C), axis=-1)    # [S, S]
    out[b] = silu(attn @ v).T.reshape(C, H, W)

Sharding: data-parallel over B across 8 cores (4 batches/core); the three
CxC projection weights are replicated.

Device-side layout choices (all matmuls in fp32r, full-rate fp32):
  - x[b] stays in its native [C, S] layout; it directly serves as the
    matmul rhs for qT/kT ([d, s] layouts) and as lhsT for v ([t, d]).
  - scores are computed transposed, scoresT[t, s], so softmax's exp is a
    plain elementwise ACT op and the softmax *sum* over t is obtained via
    an extra all-ones column appended to v (v_ext[:, 256] == 1): the
    attn@v matmul then yields the denominator as column 256 of its own
    output.  No max-subtraction is needed: |logits| <= ~8 here, exp is
    safe in fp32.
  - v's bias (and the ones column) is added via a K=1 accumulating matmul
    with an all-ones lhsT row, avoiding any partition-broadcast ops.
  - final out tile [s, d] = silu(psum[:, :256] * recip(psum[:, 256]))
    fuses normalization into the SiLU activation's per-partition scale.
The [B, S, C] device output is transposed to [B, C, H, W] on the host.
"""

import numpy as np

import concourse.bass as bass
import concourse.tile as tile
from concourse import bacc, mybir
from concourse.bass_utils import run_bass_kernel_spmd

F32 = mybir.dt.float32
F32R = mybir.dt.float32r
AF = mybir.ActivationFunctionType

B, C, H, W = 32, 256, 32, 32
S = H * W              # 1024
N_CORES = 8
BPC = B // N_CORES     # 4 batches per core
CT = C // 128          # 2 contraction tiles
DT = C // 128          # 2 output-channel tiles
TT = S // 128          # 8 key/query row tiles
NS = S // 512          # 2 512-wide column chunks
SCALE = 1.0 / 16.0     # 1/sqrt(C)


def _build_attention_core(iters=1):
    nc = bacc.Bacc("TRN2", debug=False)

    x_d = nc.dram_tensor("x", [BPC, C, S], F32R, kind="ExternalInput")
    wq_d = nc.dram_tensor("wq_t", [C, C], F32R, kind="ExternalInput")
    wk_d = nc.dram_tensor("wk_t", [C, C], F32R, kind="ExternalInput")
    wv_d = nc.dram_tensor("wv_e", [C, C + 2], F32R, kind="ExternalInput")
    bv_d = nc.dram_tensor("bv_e", [1, C + 2], F32R, kind="ExternalInput")
    ones_d = nc.dram_tensor("ones", [1, 128], F32R, kind="ExternalInput")
    bq_d = nc.dram_tensor("bq_p", [128, DT], F32, kind="ExternalInput")
    bk_d = nc.dram_tensor("bk_p", [128, DT], F32, kind="ExternalInput")
    out_d = nc.dram_tensor("out", [BPC, S, C], F32, kind="ExternalOutput")

    with tile.TileContext(nc) as tc:
        with (
            tc.tile_pool(name="consts", bufs=1) as consts,
            tc.tile_pool(name="xp", bufs=2) as xp,
            tc.tile_pool(name="qp", bufs=2) as qp,
            tc.tile_pool(name="kp", bufs=2) as kp,
            tc.tile_pool(name="vp", bufs=2) as vp,
            tc.tile_pool(name="ep", bufs=2) as ep,
            tc.tile_pool(name="op", bufs=4) as op,
            tc.tile_pool(name="ps_qk", bufs=2, space="PSUM") as ps_qk,
            tc.tile_pool(name="ps_v", bufs=2, space="PSUM") as ps_v,
            tc.tile_pool(name="ps_s", bufs=2, space="PSUM") as ps_s,
            tc.tile_pool(name="ps_o", bufs=2, space="PSUM") as ps_o,
        ):
            wq_sb = consts.tile([128, CT, C], F32R)
            wk_sb = consts.tile([128, CT, C], F32R)
            wv_sb = consts.tile([128, CT, C + 2], F32R)
            bv_sb = consts.tile([1, C + 2], F32R)
            ones_sb = consts.tile([1, 128], F32R)
            bq_sb = consts.tile([128, DT], F32)
            bk_sb = consts.tile([128, DT], F32)
            nc.sync.dma_start(out=wq_sb, in_=wq_d.ap().rearrange("(ct p) d -> p ct d", p=128))
            nc.sync.dma_start(out=wk_sb, in_=wk_d.ap().rearrange("(ct p) d -> p ct d", p=128))
            nc.sync.dma_start(out=wv_sb, in_=wv_d.ap().rearrange("(ct p) d -> p ct d", p=128))
            nc.sync.dma_start(out=bv_sb, in_=bv_d.ap())
            nc.sync.dma_start(out=ones_sb, in_=ones_d.ap())
            nc.sync.dma_start(out=bq_sb, in_=bq_d.ap())
            nc.sync.dma_start(out=bk_sb, in_=bk_d.ap())

            for b in range(BPC * iters):
                b = b % BPC
                # ---- load x[b] as [p, ct, s] (c-major, native layout) ----
                x_sb = xp.tile([128, CT, S], F32R, name=f"x_{b}", tag="x")
                for ct in range(CT):
                    nc.sync.dma_start(
                        out=x_sb[:, ct, :],
                        in_=x_d.ap()[b, ct * 128:(ct + 1) * 128, :],
                    )

                # ---- qT[d, s], kT[d, s] = w.T-slices @ x  (+ bias per-partition) ----
                q_sb = qp.tile([128, DT, S], F32R, name=f"q_{b}", tag="q")
                k_sb = kp.tile([128, DT, S], F32R, name=f"k_{b}", tag="k")
                for w_sb, b_sb, dst in ((wq_sb, bq_sb, q_sb), (wk_sb, bk_sb, k_sb)):
                    for dt in range(DT):
                        for n in range(NS):
                            pqk = ps_qk.tile([128, 512], F32, name="pqk", tag="pqk")
                            for ct in range(CT):
                                nc.tensor.matmul(
                                    pqk,
                                    w_sb[:, ct, dt * 128:(dt + 1) * 128],
                                    x_sb[:, ct, n * 512:(n + 1) * 512],
                                    start=(ct == 0),
                                    stop=(ct == CT - 1),
                                )
                            nc.vector.tensor_scalar_add(
                                dst[:, dt, n * 512:(n + 1) * 512], pqk,
                                b_sb[:, dt:dt + 1],
                            )

                # ---- v_ext[t, d'] = x-slices.T @ wv_e + ones.T @ bv_e ----
                v_sb = vp.tile([128, TT, C + 2], F32R, name=f"v_{b}", tag="v")
                for tt in range(TT):
                    pv = ps_v.tile([128, C + 2], F32, name="pv", tag="pv")
                    for ct in range(CT):
                        nc.tensor.matmul(
                            pv,
                            x_sb[:, ct, tt * 128:(tt + 1) * 128],
                            wv_sb[:, ct, :],
                            start=(ct == 0),
                            stop=False,
                        )
                    nc.tensor.matmul(
                        pv, ones_sb[:1, :], bv_sb[:1, :], start=False, stop=True,
                    )
                    nc.vector.tensor_copy(v_sb[:, tt, :], pv)

                # ---- E[t, s] = exp(scale * kT-slices.T @ qT) ----
                e_sb = ep.tile([128, TT, S], F32R, name=f"e_{b}", tag="e")
                for tt in range(TT):
                    for n in range(NS):
                        pss = ps_s.tile([128, 512], F32, name="pss", tag="pss")
                        for dt in range(DT):
                            nc.tensor.matmul(
                                pss,
                                k_sb[:, dt, tt * 128:(tt + 1) * 128],
                                q_sb[:, dt, n * 512:(n + 1) * 512],
                                start=(dt == 0),
                                stop=(dt == DT - 1),
                            )
                        nc.scalar.activation(
                            e_sb[:, tt, n * 512:(n + 1) * 512], pss, AF.Exp,
                            scale=SCALE,
                        )

                # ---- out[s, d] = silu((E-slices.T @ v_ext)[:, :256] / denom) ----
                for st in range(TT):
                    po = ps_o.tile([128, C + 2], F32, name="po", tag="po")
                    for tt in range(TT):
                        nc.tensor.matmul(
                            po,
                            e_sb[:, tt, st * 128:(st + 1) * 128],
                            v_sb[:, tt, :],
                            start=(tt == 0),
                            stop=(tt == TT - 1),
                        )
                    rec = op.tile([128, 1], F32, name="rec", tag="rec")
                    nc.vector.reciprocal(rec, po[:, C:C + 1])
                    o_sb = op.tile([128, C], F32, name="o_sb", tag="o")
                    nc.scalar.activation(o_sb, po[:, :C], AF.Silu, scale=rec)
                    nc.sync.dma_start(
                        out=out_d.ap()[b, st * 128:(st + 1) * 128, :], in_=o_sb,
                    )

    nc.compile()
    return nc


_NC_CACHE = None


def _get_nc():
    global _NC_CACHE
    if _NC_CACHE is None:
        _NC_CACHE = _build_attention_core()
    return _NC_CACHE


def _make_in_maps(x, wq, bq, wk, bk, wv, bv):
    x = np.ascontiguousarray(x, dtype=np.float32).reshape(B, C, S)
    wq_t = np.ascontiguousarray(wq.T, dtype=np.float32)
    wk_t = np.ascontiguousarray(wk.T, dtype=np.float32)
    wv_e = np.zeros((C, C + 2), dtype=np.float32)
    wv_e[:, :C] = wv.T
    bv_e = np.zeros((1, C + 2), dtype=np.float32)
    bv_e[0, C] = 1.0
    bv_e[0, :C] = bv
    ones = np.ones((1, 128), dtype=np.float32)
    bq_p = np.ascontiguousarray(bq.reshape(DT, 128).T)
    bk_p = np.ascontiguousarray(bk.reshape(DT, 128).T)
    shared = {
        "wq_t": wq_t, "wk_t": wk_t, "wv_e": wv_e, "bv_e": bv_e,
        "ones": ones, "bq_p": bq_p, "bk_p": bk_p,
    }
    return [
        {"x": x[i * BPC:(i + 1) * BPC], **shared} for i in range(N_CORES)
    ]


def kernel(x, wq, bq, wk, bk, wv, bv, _trace=False):
    nc = _get_nc()
    in_maps = _make_in_maps(
        np.asarray(x), np.asarray(wq), np.asarray(bq), np.asarray(wk),
        np.asarray(bk), np.asarray(wv), np.asarray(bv),
    )
    res = run_bass_kernel_spmd(nc, in_maps, list(range(N_CORES)), trace=_trace)
    out = np.concatenate([res.results[i]["out"] for i in range(N_CORES)], axis=0)
    out = out.transpose(0, 2, 1).reshape(B, C, H, W)
    if _trace:
        return np.ascontiguousarray(out, dtype=np.float32), res
    return np.ascontiguousarray(out, dtype=np.float32)
